# revision 4
# baseline (speedup 1.0000x reference)
import sys
sys.path.insert(0, '/opt/trn_rl_repo')
import numpy as np
import concourse.bass as bass
import concourse.mybir as mybir
from concourse import bass_utils

# Problem constants (hardcoded per contract)
B_TOTAL = 524288
NCORES = 8
BC = B_TOTAL // NCORES          # 65536 items per core
P = 128                          # partitions
CHUNKS = 4
G = BC // (CHUNKS * P)           # 128 groups per partition per chunk
DT = 0.02
GRAV = 10.0
THRUST = 6.0

F32 = mybir.dt.float32
MULT = mybir.AluOpType.mult
ADD = mybir.AluOpType.add
SUB = mybir.AluOpType.subtract
MAXOP = mybir.AluOpType.max
MINOP = mybir.AluOpType.min

_cache = {}


def _build_psd_np(log_diag, off_diag, dim):
    rows, cols = np.tril_indices(dim, -1)
    L = np.zeros((dim, dim), dtype=np.float64)
    L[np.arange(dim), np.arange(dim)] = np.exp(log_diag.astype(np.float64))
    L[rows, cols] = off_diag.astype(np.float64)
    M = L @ L.T + 1e-6 * np.eye(dim)
    return M.astype(np.float32)


def _reg_const(nc, value):
    t = nc.alloc_sbuf_tensor(f"constf32-{value}", [128, 1], F32)
    nc.gpsimd.memset(t.ap(), value)
    nc.const_aps.aps[(F32, value)] = t.ap()


def _build_nc(Q, R):
    nc = bass.Bass(trn_type="TRN2")
    _reg_const(nc, float(np.pi / 2))
    nc.all_engine_barrier()

    z_d = nc.dram_tensor("z", [BC, 3], F32, kind="ExternalInput")
    u_d = nc.dram_tensor("u", [BC, 2], F32, kind="ExternalInput")
    x_d = nc.dram_tensor("x", [BC, 6], F32, kind="ExternalInput")
    p_d = nc.dram_tensor("p", [BC, 36], F32, kind="ExternalInput")
    q_d = nc.dram_tensor("q", [128, 36], F32, kind="ExternalInput")
    xo_d = nc.dram_tensor("xo", [BC, 6], F32, kind="ExternalOutput")
    po_d = nc.dram_tensor("po", [BC, 36], F32, kind="ExternalOutput")

    def chv(t, w):
        return t.rearrange("(c p g) w -> c p (g w)", c=CHUNKS, p=P)

    zv, uv, xv, pv = chv(z_d, 3), chv(u_d, 2), chv(x_d, 6), chv(p_d, 36)
    xov, pov = chv(xo_d, 6), chv(po_d, 36)

    def sb(name, w):
        return nc.alloc_sbuf_tensor(name, [P, G * w], F32)

    # double-buffered I/O tiles
    zt = [sb(f"zt{i}", 3) for i in range(2)]
    ut = [sb(f"ut{i}", 2) for i in range(2)]
    xt = [sb(f"xt{i}", 6) for i in range(2)]
    pt = [sb(f"pt{i}", 36) for i in range(2)]
    xo = [sb(f"xot{i}", 6) for i in range(2)]
    po = [sb(f"pot{i}", 36) for i in range(2)]
    sc_s = [sb(f"sins{i}", 1) for i in range(2)]
    sc_c = [sb(f"coss{i}", 1) for i in range(2)]
    qt = nc.alloc_sbuf_tensor("qt", [128, 36], F32)

    # single-buffered scratch (vector-engine private)
    M1 = sb("M1", 36)
    PP = sb("PP", 36)
    scal = sb("scal", 8)      # mp,a,b,t0,t1,det,rdet,nrdet
    s6 = sb("s6", 6)
    cof = sb("cof", 6)
    si9 = sb("si9", 9)
    kt = sb("kt", 18)
    r6a = sb("r6a", 6)
    r6b = sb("r6b", 6)
    y3 = sb("y3", 3)

    def pl(buf, w, e, n=1):
        return buf.ap().rearrange("p (g w) -> p g w", w=w)[:, :, e:e + n]

    def bc(view, n):
        return view.broadcast_to([P, G, n])

    def flat(buf):
        return buf.ap()

    OBS = (0, 1, 4)

    with (
        nc.semaphore() as dma_in,
        nc.semaphore() as dma_out,
        nc.semaphore() as act_sem,
        nc.semaphore() as dve_done,
        nc.Block() as block,
    ):
        @block.sync
        def _(sync):
            for k in range(CHUNKS):
                bi = k % 2
                if k >= 2:
                    sync.wait_ge(dve_done, k - 1)
                sync.dma_start(zt[bi].ap(), zv[k]).then_inc(dma_in, 16)
                sync.dma_start(ut[bi].ap(), uv[k]).then_inc(dma_in, 16)
                sync.dma_start(xt[bi].ap(), xv[k]).then_inc(dma_in, 16)
                sync.dma_start(pt[bi].ap(), pv[k]).then_inc(dma_in, 16)
                if k == 0:
                    sync.dma_start(qt.ap(), q_d[:, :]).then_inc(dma_in, 16)
            for k in range(CHUNKS):
                bi = k % 2
                sync.wait_ge(dve_done, k + 1)
                sync.dma_start(xov[k], xo[bi].ap()).then_inc(dma_out, 16)
                sync.dma_start(pov[k], po[bi].ap()).then_inc(dma_out, 16)

        @block.scalar
        def _(scalar):
            for k in range(CHUNKS):
                bi = k % 2
                scalar.wait_ge(dma_in, 64 * (k + 1) + 16)
                th = pl(xt[bi], 6, 4)
                nc.scalar.activation(pl(sc_s[bi], 1, 0), th,
                                     mybir.ActivationFunctionType.Sin,
                                     bias=0.0, scale=1.0)
                nc.scalar.activation(pl(sc_c[bi], 1, 0), th,
                                     mybir.ActivationFunctionType.Sin,
                                     bias=float(np.pi / 2),
                                     scale=1.0).then_inc(act_sem, 1)

        @block.vector
        def _(vector):
            V = nc.vector
            V.memset(flat(M1), 0.0)   # row 5 stays zero forever
            for k in range(CHUNKS):
                bi = k % 2
                vector.wait_ge(dma_in, 64 * (k + 1) + 16)
                if k >= 2:
                    vector.wait_ge(dma_out, 32 * (k - 1))
                vector.wait_ge(act_sem, k + 1)

                mp = pl(scal, 8, 0)
                a = pl(scal, 8, 1)
                b = pl(scal, 8, 2)
                s_ = pl(sc_s[bi], 1, 0)
                c_ = pl(sc_c[bi], 1, 0)
                # mp = clip(u0,0,1); a = -0.12*mp*cos; b = -0.12*mp*sin
                V.tensor_scalar(mp, pl(ut[bi], 2, 0), 0.0, 1.0, MAXOP, MINOP)
                V.scalar_tensor_tensor(a, mp, -THRUST * DT, c_, MULT, MULT)
                V.scalar_tensor_tensor(b, mp, -THRUST * DT, s_, MULT, MULT)

                # ---- x_pred into xo ----
                xp = lambda e, n=1: pl(xt[bi], 6, e, n)
                xq = lambda e, n=1: pl(xo[bi], 6, e, n)
                V.tensor_tensor(xq(2), xp(2), b, ADD)                 # nvx
                V.scalar_tensor_tensor(xq(3), a, -1.0, xp(3), MULT, ADD)
                V.tensor_scalar_add(xq(3), xq(3), -GRAV * DT)         # nvy
                V.scalar_tensor_tensor(xq(5), pl(ut[bi], 2, 1), DT, xp(5), MULT, ADD)
                V.scalar_tensor_tensor(xq(0), xq(2), DT, xp(0), MULT, ADD)
                V.scalar_tensor_tensor(xq(1), xq(3), DT, xp(1), MULT, ADD)
                V.scalar_tensor_tensor(xq(4), xq(5), DT, xp(4), MULT, ADD)

                # ---- P_pred ----
                prow = lambda i: pl(pt[bi], 36, 6 * i, 6)
                m1row = lambda i: pl(M1, 36, 6 * i, 6)
                m1col = lambda kk: M1.ap().rearrange(
                    "p (g r c) -> p g r c", r=6, c=6)[:, :, :, kk]
                pprow = lambda i: pl(PP, 36, 6 * i, 6)
                ppcol = lambda kk: PP.ap().rearrange(
                    "p (g r c) -> p g r c", r=6, c=6)[:, :, :, kk]
                a6 = bc(a, 6)
                b6 = bc(b, 6)
                V.tensor_tensor(m1row(2), a6, prow(4), MULT)
                V.tensor_tensor(m1row(3), b6, prow(4), MULT)
                V.tensor_tensor(m1row(0), prow(2), m1row(2), ADD)
                V.tensor_scalar_mul(m1row(0), m1row(0), DT)
                V.tensor_tensor(m1row(1), prow(3), m1row(3), ADD)
                V.tensor_scalar_mul(m1row(1), m1row(1), DT)
                V.tensor_scalar_mul(m1row(4), prow(5), DT)
                # PP = P + M1 + Q
                V.tensor_tensor(flat(PP), flat(pt[bi]), flat(M1), ADD)
                qb = qt.ap().unsqueeze(1).broadcast_to([P, G, 36])
                ppg = PP.ap().rearrange("p (g w) -> p g w", w=36)
                V.tensor_tensor(ppg, ppg, qb, ADD)
                # PP += M1^T  (6 row ops: PP row i += M1 col i)
                for i in range(6):
                    V.tensor_tensor(pprow(i), pprow(i), m1col(i), ADD)
                # PP += E * M1^T
                ra = pl(r6a, 6, 0, 6)
                rb = pl(r6b, 6, 0, 6)
                V.tensor_tensor(ra, a6, m1col(4), MULT)       # T row2
                V.tensor_tensor(pprow(2), pprow(2), ra, ADD)
                V.tensor_tensor(rb, b6, m1col(4), MULT)       # T row3
                V.tensor_tensor(pprow(3), pprow(3), rb, ADD)
                V.tensor_tensor(ra, m1col(2), ra, ADD)        # row0 pre
                V.tensor_scalar_mul(ra, ra, DT)
                V.tensor_tensor(pprow(0), pprow(0), ra, ADD)
                V.tensor_tensor(rb, m1col(3), rb, ADD)        # row1 pre
                V.tensor_scalar_mul(rb, rb, DT)
                V.tensor_tensor(pprow(1), pprow(1), rb, ADD)
                V.tensor_scalar_mul(ra, m1col(5), DT)         # row4
                V.tensor_tensor(pprow(4), pprow(4), ra, ADD)

                # ---- S = PP[obs,obs] + R ; inverse via adjugate ----
                ppe = lambda i, j: pl(PP, 36, 6 * i + j)
                sid = {(0, 0): 0, (0, 1): 1, (0, 2): 2, (1, 1): 3, (1, 2): 4, (2, 2): 5}
                se = lambda m, n2: pl(s6, 6, sid[(min(m, n2), max(m, n2))])
                for (m, n2), idx in sid.items():
                    V.tensor_scalar_add(pl(s6, 6, idx),
                                        ppe(OBS[m], OBS[n2]), float(R[m, n2]))
                t0 = pl(scal, 8, 3)
                t1 = pl(scal, 8, 4)
                det = pl(scal, 8, 5)
                rdet = pl(scal, 8, 6)
                cofs = [((1, 1), (2, 2), (1, 2), (1, 2)),   # c00
                        ((0, 2), (1, 2), (0, 1), (2, 2)),   # c01
                        ((0, 1), (1, 2), (0, 2), (1, 1)),   # c02
                        ((0, 0), (2, 2), (0, 2), (0, 2)),   # c11
                        ((0, 1), (0, 2), (0, 0), (1, 2)),   # c12
                        ((0, 0), (1, 1), (0, 1), (0, 1))]   # c22
                for idx, (p1, p2, p3, p4) in enumerate(cofs):
                    V.tensor_tensor(t0, se(*p1), se(*p2), MULT)
                    V.tensor_tensor(t1, se(*p3), se(*p4), MULT)
                    V.tensor_tensor(pl(cof, 6, idx), t0, t1, SUB)
                V.tensor_tensor(det, se(0, 0), pl(cof, 6, 0), MULT)
                V.tensor_tensor(t0, se(0, 1), pl(cof, 6, 1), MULT)
                V.tensor_tensor(det, det, t0, ADD)
                V.tensor_tensor(t0, se(0, 2), pl(cof, 6, 2), MULT)
                V.tensor_tensor(det, det, t0, ADD)
                V.reciprocal(rdet, det)
                nrdet = pl(scal, 8, 7)
                V.tensor_scalar_mul(nrdet, rdet, -1.0)
                # si9 = -Sinv full 3x3 (row-major); cof order 00,01,02,11,12,22
                cmap = [0, 1, 2, 1, 3, 4, 2, 4, 5]
                V.tensor_tensor(pl(cof, 6, 0, 6), pl(cof, 6, 0, 6),
                                bc(nrdet, 6), MULT)
                for e9, c6 in enumerate(cmap):
                    V.tensor_copy(pl(si9, 9, e9), pl(cof, 6, c6))

                # ---- y = z - x_pred[obs] ----
                V.tensor_tensor(pl(y3, 3, 0, 2), pl(zt[bi], 3, 0, 2),
                                pl(xo[bi], 6, 0, 2), SUB)
                V.tensor_tensor(pl(y3, 3, 2), pl(zt[bi], 3, 2),
                                pl(xo[bi], 6, 4), SUB)

                # ---- KT = (-Sinv) @ U^T   (3x6), U col m = PP col OBS[m]
                for m in range(3):
                    krow = pl(kt, 18, 6 * m, 6)
                    V.tensor_tensor(krow, bc(pl(si9, 9, 3 * m), 6),
                                    ppcol(OBS[0]), MULT)
                    for n2 in (1, 2):
                        V.tensor_tensor(ra, bc(pl(si9, 9, 3 * m + n2), 6),
                                        ppcol(OBS[n2]), MULT)
                        V.tensor_tensor(krow, krow, ra, ADD)

                # ---- x_upd = x_pred - KT^T y ----
                for m in range(3):
                    V.tensor_tensor(ra, pl(kt, 18, 6 * m, 6),
                                    bc(pl(y3, 3, m), 6), MULT)
                    V.tensor_tensor(pl(xo[bi], 6, 0, 6),
                                    pl(xo[bi], 6, 0, 6), ra, SUB)

                # ---- P_upd = PP + sum_m outer(KT[m,:] (i), U[:,m] (j)) ----
                porow = lambda i: pl(po[bi], 36, 6 * i, 6)
                last = None
                for i in range(6):
                    first = True
                    for m in range(3):
                        ktmi = pl(kt, 18, 6 * m + i)
                        V.tensor_tensor(ra, bc(ktmi, 6), ppcol(OBS[m]), MULT)
                        if first:
                            last = V.tensor_tensor(porow(i), pprow(i), ra, ADD)
                            first = False
                        else:
                            last = V.tensor_tensor(porow(i), porow(i), ra, ADD)

                last.then_inc(dve_done, 1)
    return nc


def kernel(z, u, x_prev, P_prev, q_log_diag, q_off_diag, r_log_diag, r_off_diag):
    Q = _build_psd_np(np.asarray(q_log_diag), np.asarray(q_off_diag), 6)
    R = _build_psd_np(np.asarray(r_log_diag), np.asarray(r_off_diag), 3)
    key = (Q.tobytes(), R.tobytes())
    if "nc" not in _cache or _cache.get("key") != key:
        _cache["nc"] = _build_nc(Q, R)
        _cache["key"] = key
    nc = _cache["nc"]

    z = np.ascontiguousarray(np.asarray(z, dtype=np.float32))
    u = np.ascontiguousarray(np.asarray(u, dtype=np.float32))
    x_prev = np.ascontiguousarray(np.asarray(x_prev, dtype=np.float32))
    Pp = np.ascontiguousarray(
        np.asarray(P_prev, dtype=np.float32).reshape(B_TOTAL, 36))
    qrep = np.tile(Q.reshape(1, 36), (128, 1)).astype(np.float32)

    in_maps = []
    for c in range(NCORES):
        s, e = c * BC, (c + 1) * BC
        in_maps.append({
            "z": z[s:e], "u": u[s:e], "x": x_prev[s:e], "p": Pp[s:e],
            "q": qrep,
        })
    res = bass_utils.run_bass_kernel_spmd(nc, in_maps, core_ids=list(range(NCORES)))
    x_out = np.concatenate([res.results[c]["xo"] for c in range(NCORES)], axis=0)
    p_out = np.concatenate([res.results[c]["po"] for c in range(NCORES)], axis=0)
    return x_out, p_out.reshape(B_TOTAL, 6, 6)
